# revision 30
# baseline (speedup 1.0000x reference)
"""Trainium2 Bass kernel for batched Gaussian log-density quadratic form.

Computes out = -einsum('nd,de,ne->n', Y, prec, Y) with Y = X - mean,
X: [65536, 256] f32, mean: [1, 256] f32, prec: [256, 256] f32.

Strategy (data-parallel over rows, 8 NeuronCores), transposed layout:
  Only the symmetric part S = (P + P^T)/2 matters.  Host factors
      S = A diag(w) A^T
  with A from block-Schur + per-block eigendecomposition so that
  A[0:128, 128:256] == 0 (3 nonzero 128x128 blocks -> 3 matmuls per
  512-column sub-block).  Column k of chunk0 is paired with column k of
  chunk1 and both columns rescaled so the two carry the SAME reduce
  weight w*_k = sign * sqrt(|w_a w_b|) (same-sign pairs add, the
  leftover mixed-sign pair subtracts).  Device, per 512-col sub-block:
      V  = A^T y        3 accumulating fp16 matmuls             (PE)
      Sq = V^2          Square, PSUM f32 -> SBUF f16        (ACT/DVE)
      M  = Sq0 +- Sq1   chunk merge                        (DVE/Pool)
      out = w*^T M      ONE reduce-matmul, row parked at a
                        32-aligned partition of a PSUM O tile    (PE)
  O tiles drain via strided-partition copies + strided DMA stores.
  y is fp16 (half HBM traffic); all matmuls run 1 cycle/row.  Dummy
  warm-up matmuls ramp the PE p-state while the first y DMA flies.
"""

import numpy as np

N, D = 65536, 256
N_CORES = 8
NS = N // N_CORES  # 8192 rows per core
P = 128
SB = 512  # matmul free size / sub-block columns
NSB = NS // SB  # 16 sub-blocks per core
BLK = 1024  # columns per steady-state y DMA
NBLK = NS // BLK  # 8
RLAG = 5  # sub-blocks between V matmuls and their reduce matmul
NWARM = 11  # dummy warm-up matmuls (free=256) to ramp the PE p-state
DVE_SQ = (2, 4, 7, 9, 11)  # sub-blocks whose Square runs on DVE (rest: ACT)
POOL_MERGE = (1, 4, 7, 10)  # sub-blocks whose merge (add+sub) runs on GpSimd
DRAIN_ACT = (0, 2, 3)  # O tiles drained on ACT (t3a counts as tile 3); rest DVE

TRACE = False
LAST_EXEC_NS = None
LAST_RESULTS = None

_PROGRAMS = {}
_BUILD_KEY = ("schur", P)  # set by _host_inputs: (variant, nadd)


def _build_program(key):
    import concourse.bass as bass
    import concourse.tile as tile
    from concourse import bacc, mybir
    from contextlib import ExitStack

    variant, nsub = key
    F32 = mybir.dt.float32
    F16 = mybir.dt.float16
    NMM = 3 if variant == "schur" else 4
    AW = NMM * P + 3  # a chunks + wstar + per-chunk weight columns

    nc = bacc.Bacc("TRN2", target_bir_lowering=False, debug=False)
    # y^T per core: [d-chunk, d-in-chunk, n] fp16, host pre-subtracted mean
    y_dram = nc.dram_tensor("y", [2, P, NS], F16, kind="ExternalInput").ap()
    # packed stationaries: NMM 128x128 chunks of A + the w* column
    aw_dram = nc.dram_tensor("aw", [P, AW], F16, kind="ExternalInput").ap()
    out_dram = nc.dram_tensor("out", [NS], F32, kind="ExternalOutput").ap()

    with tile.TileContext(nc) as tc, ExitStack() as ctx:
        singles = ctx.enter_context(tc.tile_pool(name="singles", bufs=1))
        ypool = ctx.enter_context(tc.tile_pool(name="ypool", bufs=4))
        sqpool = ctx.enter_context(tc.tile_pool(name="sqpool", bufs=7))
        mpool = ctx.enter_context(tc.tile_pool(name="mpool", bufs=8))
        zpool = ctx.enter_context(tc.tile_pool(name="zpool", bufs=3, space="PSUM"))
        opool = ctx.enter_context(tc.tile_pool(name="opool", bufs=2, space="PSUM"))

        aw = singles.tile([P, AW], F16)
        amat = [aw[:, j * P : (j + 1) * P] for j in range(NMM)]
        wstar = aw[:, NMM * P : NMM * P + 1]
        wcol = [aw[:, NMM * P + 1 + c : NMM * P + 2 + c] for c in range(2)]

        # PE p-state warm-up fodder
        warm = singles.tile([P, 256], F16)
        nc.vector.memset(warm, 0.0)

        otiles = {}
        stg = [
            singles.tile([P, SB], F32, tag=f"stg{t}", name=f"stg{t}") for t in range(5)
        ]

        y_view = y_dram.rearrange("c p n -> p c n")  # [128, 2, 8192]
        out_view = out_dram.rearrange("(t r j) -> t r j", t=4, r=4)

        # issue every y DMA up front on SP; pool rotation paces them.
        # first block = two pieces (256+256 then 512) so compute starts at
        # the earliest moment; aw lands right after the first piece.
        H = SB // 2
        y0p = []
        for j, (lo, hi) in enumerate(((0, H), (H, SB))):
            yt = singles.tile([P, 2, hi - lo], F16, tag=f"y0{j}", name=f"y0{j}")
            nc.sync.dma_start(yt, y_view[:, :, lo:hi])
            y0p.append(yt)
            if j == 0:
                nc.gpsimd.dma_start(aw, aw_dram)
        y0c = singles.tile([P, 2, SB], F16, tag="y0c", name="y0c")
        nc.sync.dma_start(y0c, y_view[:, :, SB:BLK])
        ysubs = [None, y0c[:, :, :]]
        for g in range(1, NBLK):
            yg = ypool.tile([P, 2, BLK], F16, tag="y", name="yg")
            nc.sync.dma_start(yg, y_view[:, :, g * BLK : (g + 1) * BLK])
            for h in range(2):
                ysubs.append(yg[:, :, h * SB : (h + 1) * SB])

        zw = zpool.tile([P, 2, SB], F32, tag="z", name="zw")
        for _ in range(NWARM):
            nc.tensor.matmul(
                zw[:, 0, 0:256], lhsT=warm[:, 0:P], rhs=warm, start=True, stop=True
            )

        def drain(t, rows, engine, okey=None, st=None, orows=None):
            # O rows {0,32,64,96}[rows] via strided-partition copy + DMA
            orows = rows if orows is None else orows
            osrc = otiles[okey if okey is not None else t]
            osrc = osrc.rearrange("(r q) j -> r q j", q=32)[orows, 0, :]
            dst = stg[st if st is not None else t]
            dst = dst.rearrange("(r q) j -> r q j", q=32)[orows, 0, :]
            engine(dst, osrc)
            nc.sync.dma_start(out_view[t, rows, :], dst)

        def emit_reduce(s, m):
            t, r = s // 4, 32 * (s % 4)
            if s == NSB - 1:
                # last sub-block gets its own 1-bank tile: no false WAR
                # against the rows-{0,32,64} drain of tile 3
                t, r = "last", 0
            if r == 0:
                otiles[t] = opool.tile([P, SB], F32, tag="o", name="o")
            o = otiles[t]
            if m is None:  # closing sub-blocks: 2-matmul reduce, no merge
                sq = sq_of[s]
                nc.tensor.matmul(
                    o[r : r + 1, :], lhsT=wcol[0], rhs=sq[:, 0, :],
                    start=True, stop=False, tile_position=(0, r),
                )
                nc.tensor.matmul(
                    o[r : r + 1, :], lhsT=wcol[1], rhs=sq[:, 1, :],
                    start=False, stop=True, tile_position=(0, r),
                )
            else:
                nc.tensor.matmul(
                    o[r : r + 1, :], lhsT=wstar, rhs=m,
                    start=True, stop=True, tile_position=(0, r),
                )
            if s == 14:
                drain(3, slice(0, 3), nc.scalar.copy)
            elif s == 15:
                osrc = otiles["last"][0:1, :]
                dst = stg[4][0:1, :]
                nc.vector.tensor_copy(dst, osrc)
                nc.sync.dma_start(out_view[3, 3:4, :], dst)
            elif s % 4 == 3:
                t = s // 4
                eng = nc.scalar.copy if t in DRAIN_ACT else nc.vector.tensor_copy
                drain(t, slice(0, 4), eng)

        pending = []
        sq_of = {}
        for s in range(NSB):
            z = zpool.tile([P, 2, SB], F32, tag="z", name="z")
            if s == 0:
                for j in range(2):
                    ha, hb = j * H, (j + 1) * H
                    p0 = y0p[j][:, 0, :]
                    p1 = y0p[j][:, 1, :]
                    nc.tensor.matmul(
                        z[:, 0, ha:hb], lhsT=amat[0], rhs=p0, start=True, stop=False
                    )
                    nc.tensor.matmul(
                        z[:, 0, ha:hb], lhsT=amat[1], rhs=p1, start=False, stop=True
                    )
                    if variant == "schur":
                        nc.tensor.matmul(
                            z[:, 1, ha:hb], lhsT=amat[2], rhs=p1,
                            start=True, stop=True,
                        )
                    else:
                        nc.tensor.matmul(
                            z[:, 1, ha:hb], lhsT=amat[2], rhs=p0,
                            start=True, stop=False,
                        )
                        nc.tensor.matmul(
                            z[:, 1, ha:hb], lhsT=amat[3], rhs=p1,
                            start=False, stop=True,
                        )
            elif variant == "schur":
                ys = ysubs[s]
                y0 = ys[:, 0, :]
                y1 = ys[:, 1, :]
                # V0 = A00^T y0 + A10^T y1 ; V1 = A11^T y1
                nc.tensor.matmul(
                    z[:, 0, :], lhsT=amat[0], rhs=y0, start=True, stop=False
                )
                nc.tensor.matmul(
                    z[:, 0, :], lhsT=amat[1], rhs=y1, start=False, stop=True
                )
                nc.tensor.matmul(
                    z[:, 1, :], lhsT=amat[2], rhs=y1, start=True, stop=True
                )
            else:
                ys = ysubs[s]
                y0 = ys[:, 0, :]
                y1 = ys[:, 1, :]
                nc.tensor.matmul(
                    z[:, 0, :], lhsT=amat[0], rhs=y0, start=True, stop=False
                )
                nc.tensor.matmul(
                    z[:, 0, :], lhsT=amat[1], rhs=y1, start=False, stop=True
                )
                nc.tensor.matmul(
                    z[:, 1, :], lhsT=amat[2], rhs=y0, start=True, stop=False
                )
                nc.tensor.matmul(
                    z[:, 1, :], lhsT=amat[3], rhs=y1, start=False, stop=True
                )
            sq = sqpool.tile([P, 2, SB], F16, tag="sq", name="sq")
            sq_of[s] = sq
            if s == NSB - 1:
                H = SB // 2
                nc.scalar.square(sq[:, :, 0:H], z[:, :, 0:H])
                nc.vector.tensor_mul(sq[:, :, H:SB], z[:, :, H:SB], z[:, :, H:SB])
            elif s in DVE_SQ:
                nc.vector.tensor_mul(sq, z, z)
            else:
                nc.scalar.square(sq, z)
            if s >= NSB - 3:
                pending.append((s, None))
            else:
                m = mpool.tile([P, SB], F16, tag="m", name="m")
                eng = nc.gpsimd if s in POOL_MERGE else nc.vector
                eng.tensor_add(m, sq[:, 0, :], sq[:, 1, :])
                if nsub:
                    # mixed-sign slots live at the front; overwrite them
                    eng.tensor_sub(
                        m[0:nsub, :], sq[0:nsub, 1, :], sq[0:nsub, 0, :]
                    )
                pending.append((s, m))
            if len(pending) > RLAG:
                emit_reduce(*pending.pop(0))
        for item in pending:
            emit_reduce(*item)

    nc.compile()
    return nc


def _get_program():
    nc = _PROGRAMS.get(_BUILD_KEY)
    if nc is None:
        nc = _PROGRAMS[_BUILD_KEY] = _build_program(_BUILD_KEY)
    return nc


def _factor(prec):
    """S = A diag(w) A^T with A[0:128, 128:256] = 0 when well-conditioned
    (schur variant, 3 matmuls), else dense eigh (4). Columns normalized."""
    S = 0.5 * (prec + prec.T)
    S00, S10, S11 = S[:P, :P], S[P:, :P], S[P:, P:]
    l0, Q0 = np.linalg.eigh(S00)
    ok = np.abs(l0).min() > 1e-3
    if ok:
        A10 = S10 @ Q0 @ np.diag(1.0 / l0)
        ok = np.abs(A10).max() < 500.0
    if ok:
        C = S11 - (A10 * l0) @ A10.T
        lc, Qc = np.linalg.eigh(C)
        A = np.zeros((D, D))
        A[:P, :P] = Q0
        A[P:, :P] = A10
        A[P:, P:] = Qc
        w = np.concatenate([l0, lc])
        variant = "schur"
    else:
        w, A = np.linalg.eigh(S)
        variant = "eigh"
    nrm = np.linalg.norm(A, axis=0)
    return variant, A / nrm, -(w * nrm**2)


def _pair(An, wn):
    """Order/rescale columns so chunk0/chunk1 slot k share |weight|; returns
    (A0, A1, wstar, wab, nadd): first nadd slots add, the rest subtract."""
    w0, w1 = wn[:P], wn[P:]
    A0, A1 = An[:, :P], An[:, P:]

    def by_sign(wv, sgn):
        idx = np.where(np.sign(wv) == sgn)[0]
        return idx[np.argsort(-np.abs(wv[idx]))]

    p0, n0 = by_sign(w0, 1), by_sign(w0, -1)
    p1, n1 = by_sign(w1, 1), by_sign(w1, -1)
    kp, kn = min(len(p0), len(p1)), min(len(n0), len(n1))
    # mixed-sign leftovers first (aligned overwrite-sub), then same-sign pairs
    s0 = np.concatenate([p0[kp:], n0[kn:], p0[:kp], n0[:kn]]).astype(int)
    s1 = np.concatenate([n1[kn:], p1[kp:], p1[:kp], n1[:kn]]).astype(int)
    nsub = len(s0) - (kp + kn)
    wa, wb = w0[s0], w1[s1]
    wstar = np.sign(wb) * np.sqrt(np.abs(wa) * np.abs(wb))
    c0 = np.sqrt(np.abs(wa) / np.abs(wstar))
    c1 = np.sqrt(np.abs(wb) / np.abs(wstar))
    # after rescaling, per-chunk weights are +-|wstar| with original signs
    wab = np.stack([np.sign(wa) * np.abs(wstar), np.sign(wb) * np.abs(wstar)], 1)
    return A0[:, s0] * c0[None, :], A1[:, s1] * c1[None, :], wstar, wab, nsub


def _host_inputs(X, mean, prec):
    global _BUILD_KEY
    X = np.ascontiguousarray(np.asarray(X, dtype=np.float32))
    m = np.asarray(mean, dtype=np.float32).reshape(-1)
    Pm = np.asarray(prec, dtype=np.float64)

    variant, An, wn = _factor(Pm)
    A0, A1, wstar, wab, nsub = _pair(An, wn)
    _BUILD_KEY = (variant, int(nsub))

    if variant == "schur":
        chunks = [A0[:P], A0[P:], A1[P:]]
    else:
        chunks = [A0[:P], A0[P:], A1[:P], A1[P:]]
    aw_host = np.concatenate(
        [np.concatenate(chunks, axis=1), wstar[:, None], wab], axis=1
    ).astype(np.float16)
    aw_host = np.ascontiguousarray(aw_host)

    Y = (X - m[None, :]).astype(np.float16)
    Yt = np.ascontiguousarray(Y.T)  # [256, 65536] fp16
    in_maps = [
        {
            "y": np.ascontiguousarray(
                Yt[:, i * NS : (i + 1) * NS].reshape(2, P, NS)
            ),
            "aw": aw_host,
        }
        for i in range(N_CORES)
    ]
    return in_maps


def kernel(X, mean, prec):
    global LAST_EXEC_NS, LAST_RESULTS
    from concourse.bass_utils import run_bass_kernel_spmd

    in_maps = _host_inputs(X, mean, prec)
    nc = _get_program()
    res = run_bass_kernel_spmd(
        nc, in_maps, core_ids=list(range(N_CORES)), trace=TRACE
    )
    LAST_RESULTS = res
    LAST_EXEC_NS = res.exec_time_ns
    out = np.concatenate([res.results[i]["out"] for i in range(N_CORES)])
    return out.astype(np.float32)


# revision 31
# speedup vs baseline: 1.0678x; 1.0678x over previous
"""Trainium2 Bass kernel for batched Gaussian log-density quadratic form.

Computes out = -einsum('nd,de,ne->n', Y, prec, Y) with Y = X - mean,
X: [65536, 256] f32, mean: [1, 256] f32, prec: [256, 256] f32.

Strategy (data-parallel over rows, 8 NeuronCores), transposed layout:
  Only the symmetric part S = (P + P^T)/2 matters.  Host factors
      S = A diag(w) A^T
  with A from block-Schur + per-block eigendecomposition so that
  A[0:128, 128:256] == 0 (3 nonzero 128x128 blocks -> 3 matmuls per
  512-column sub-block).  Column k of chunk0 is paired with column k of
  chunk1 and both columns rescaled so the two carry the SAME reduce
  weight w*_k = sign * sqrt(|w_a w_b|) (same-sign pairs add, the
  leftover mixed-sign pair subtracts).  Device, per 512-col sub-block:
      V  = A^T y        3 accumulating fp16 matmuls             (PE)
      Sq = V^2          Square, PSUM f32 -> SBUF f16        (ACT/DVE)
      M  = Sq0 +- Sq1   chunk merge                        (DVE/Pool)
      out = w*^T M      ONE reduce-matmul, row parked at a
                        32-aligned partition of a PSUM O tile    (PE)
  O tiles drain via strided-partition copies + strided DMA stores.
  y is fp16 (half HBM traffic); all matmuls run 1 cycle/row.  Dummy
  warm-up matmuls ramp the PE p-state while the first y DMA flies.
"""

import numpy as np

N, D = 65536, 256
N_CORES = 8
NS = N // N_CORES  # 8192 rows per core
P = 128
SB = 512  # matmul free size / sub-block columns
NSB = NS // SB  # 16 sub-blocks per core
BLK = 1024  # columns per steady-state y DMA
NBLK = NS // BLK  # 8
RLAG = 5  # sub-blocks between V matmuls and their reduce matmul
NWARM = 11  # dummy warm-up matmuls (free=256) to ramp the PE p-state
DVE_SQ = (2, 5, 8, 11)  # sub-blocks whose Square runs on DVE (rest: ACT)
POOL_MERGE = (1, 4, 7, 10)  # sub-blocks whose merge (add+sub) runs on GpSimd
DRAIN_ACT = (0, 2, 3)  # O tiles drained on ACT (t3a counts as tile 3); rest DVE

TRACE = False
LAST_EXEC_NS = None
LAST_RESULTS = None

_PROGRAMS = {}
_BUILD_KEY = ("schur", P)  # set by _host_inputs: (variant, nadd)


def _build_program(key):
    import concourse.bass as bass
    import concourse.tile as tile
    from concourse import bacc, mybir
    from contextlib import ExitStack

    variant, nsub = key
    F32 = mybir.dt.float32
    F16 = mybir.dt.float16
    NMM = 3 if variant == "schur" else 4
    AW = NMM * P + 3  # a chunks + wstar + per-chunk weight columns

    nc = bacc.Bacc("TRN2", target_bir_lowering=False, debug=False)
    # y^T per core: [d-chunk, d-in-chunk, n] fp16, host pre-subtracted mean
    y_dram = nc.dram_tensor("y", [2, P, NS], F16, kind="ExternalInput").ap()
    # packed stationaries: NMM 128x128 chunks of A + the w* column
    aw_dram = nc.dram_tensor("aw", [P, AW], F16, kind="ExternalInput").ap()
    out_dram = nc.dram_tensor("out", [NS], F32, kind="ExternalOutput").ap()

    with tile.TileContext(nc) as tc, ExitStack() as ctx:
        singles = ctx.enter_context(tc.tile_pool(name="singles", bufs=1))
        ypool = ctx.enter_context(tc.tile_pool(name="ypool", bufs=4))
        sqpool = ctx.enter_context(tc.tile_pool(name="sqpool", bufs=7))
        mpool = ctx.enter_context(tc.tile_pool(name="mpool", bufs=8))
        zpool = ctx.enter_context(tc.tile_pool(name="zpool", bufs=3, space="PSUM"))
        opool = ctx.enter_context(tc.tile_pool(name="opool", bufs=2, space="PSUM"))

        aw = singles.tile([P, AW], F16)
        amat = [aw[:, j * P : (j + 1) * P] for j in range(NMM)]
        wstar = aw[:, NMM * P : NMM * P + 1]
        wcol = [aw[:, NMM * P + 1 + c : NMM * P + 2 + c] for c in range(2)]

        # PE p-state warm-up fodder
        warm = singles.tile([P, 256], F16)
        nc.vector.memset(warm, 0.0)

        otiles = {}
        stg = [
            singles.tile([P, SB], F32, tag=f"stg{t}", name=f"stg{t}") for t in range(5)
        ]

        y_view = y_dram.rearrange("c p n -> p c n")  # [128, 2, 8192]
        out_view = out_dram.rearrange("(t r j) -> t r j", t=4, r=4)

        # issue every y DMA up front on SP; pool rotation paces them.
        # first block = two pieces (256+256 then 512) so compute starts at
        # the earliest moment; aw lands right after the first piece.
        H = SB // 2
        y0p = []
        for j, (lo, hi) in enumerate(((0, H), (H, SB))):
            yt = singles.tile([P, 2, hi - lo], F16, tag=f"y0{j}", name=f"y0{j}")
            nc.sync.dma_start(yt, y_view[:, :, lo:hi])
            y0p.append(yt)
            if j == 0:
                nc.gpsimd.dma_start(aw, aw_dram)
        y0c = singles.tile([P, 2, SB], F16, tag="y0c", name="y0c")
        nc.sync.dma_start(y0c, y_view[:, :, SB:BLK])
        ysubs = [None, y0c[:, :, :]]
        for g in range(1, NBLK):
            yg = ypool.tile([P, 2, BLK], F16, tag="y", name="yg")
            nc.sync.dma_start(yg, y_view[:, :, g * BLK : (g + 1) * BLK])
            for h in range(2):
                ysubs.append(yg[:, :, h * SB : (h + 1) * SB])

        zw = zpool.tile([P, 2, SB], F32, tag="z", name="zw")
        for _ in range(NWARM):
            nc.tensor.matmul(
                zw[:, 0, 0:256], lhsT=warm[:, 0:P], rhs=warm, start=True, stop=True
            )

        def drain(t, rows, engine, okey=None, st=None, orows=None):
            # O rows {0,32,64,96}[rows] via strided-partition copy + DMA
            orows = rows if orows is None else orows
            osrc = otiles[okey if okey is not None else t]
            osrc = osrc.rearrange("(r q) j -> r q j", q=32)[orows, 0, :]
            dst = stg[st if st is not None else t]
            dst = dst.rearrange("(r q) j -> r q j", q=32)[orows, 0, :]
            engine(dst, osrc)
            nc.sync.dma_start(out_view[t, rows, :], dst)

        def emit_reduce(s, m):
            t, r = s // 4, 32 * (s % 4)
            if s == NSB - 1:
                # last sub-block gets its own 1-bank tile: no false WAR
                # against the rows-{0,32,64} drain of tile 3
                t, r = "last", 0
            if r == 0:
                otiles[t] = opool.tile([P, SB], F32, tag="o", name="o")
            o = otiles[t]
            if m is None:  # closing sub-blocks: 2-matmul reduce, no merge
                sq = sq_of[s]
                nc.tensor.matmul(
                    o[r : r + 1, :], lhsT=wcol[0], rhs=sq[:, 0, :],
                    start=True, stop=False, tile_position=(0, r),
                )
                nc.tensor.matmul(
                    o[r : r + 1, :], lhsT=wcol[1], rhs=sq[:, 1, :],
                    start=False, stop=True, tile_position=(0, r),
                )
            else:
                nc.tensor.matmul(
                    o[r : r + 1, :], lhsT=wstar, rhs=m,
                    start=True, stop=True, tile_position=(0, r),
                )
            if s == 14:
                drain(3, slice(0, 3), nc.scalar.copy)
            elif s == 15:
                osrc = otiles["last"][0:1, :]
                dst = stg[4][0:1, :]
                nc.vector.tensor_copy(dst, osrc)
                nc.sync.dma_start(out_view[3, 3:4, :], dst)
            elif s % 4 == 3:
                t = s // 4
                eng = nc.scalar.copy if t in DRAIN_ACT else nc.vector.tensor_copy
                drain(t, slice(0, 4), eng)

        pending = []
        sq_of = {}
        for s in range(NSB):
            z = zpool.tile([P, 2, SB], F32, tag="z", name="z")
            if s == 0:
                for j in range(2):
                    ha, hb = j * H, (j + 1) * H
                    p0 = y0p[j][:, 0, :]
                    p1 = y0p[j][:, 1, :]
                    nc.tensor.matmul(
                        z[:, 0, ha:hb], lhsT=amat[0], rhs=p0, start=True, stop=False
                    )
                    nc.tensor.matmul(
                        z[:, 0, ha:hb], lhsT=amat[1], rhs=p1, start=False, stop=True
                    )
                    if variant == "schur":
                        nc.tensor.matmul(
                            z[:, 1, ha:hb], lhsT=amat[2], rhs=p1,
                            start=True, stop=True,
                        )
                    else:
                        nc.tensor.matmul(
                            z[:, 1, ha:hb], lhsT=amat[2], rhs=p0,
                            start=True, stop=False,
                        )
                        nc.tensor.matmul(
                            z[:, 1, ha:hb], lhsT=amat[3], rhs=p1,
                            start=False, stop=True,
                        )
            elif variant == "schur":
                ys = ysubs[s]
                y0 = ys[:, 0, :]
                y1 = ys[:, 1, :]
                # V0 = A00^T y0 + A10^T y1 ; V1 = A11^T y1
                nc.tensor.matmul(
                    z[:, 0, :], lhsT=amat[0], rhs=y0, start=True, stop=False
                )
                nc.tensor.matmul(
                    z[:, 0, :], lhsT=amat[1], rhs=y1, start=False, stop=True
                )
                nc.tensor.matmul(
                    z[:, 1, :], lhsT=amat[2], rhs=y1, start=True, stop=True
                )
            else:
                ys = ysubs[s]
                y0 = ys[:, 0, :]
                y1 = ys[:, 1, :]
                nc.tensor.matmul(
                    z[:, 0, :], lhsT=amat[0], rhs=y0, start=True, stop=False
                )
                nc.tensor.matmul(
                    z[:, 0, :], lhsT=amat[1], rhs=y1, start=False, stop=True
                )
                nc.tensor.matmul(
                    z[:, 1, :], lhsT=amat[2], rhs=y0, start=True, stop=False
                )
                nc.tensor.matmul(
                    z[:, 1, :], lhsT=amat[3], rhs=y1, start=False, stop=True
                )
            sq = sqpool.tile([P, 2, SB], F16, tag="sq", name="sq")
            sq_of[s] = sq
            if s == NSB - 1:
                H = SB // 2
                nc.scalar.square(sq[:, :, 0:H], z[:, :, 0:H])
                nc.vector.tensor_mul(sq[:, :, H:SB], z[:, :, H:SB], z[:, :, H:SB])
            elif s in DVE_SQ:
                nc.vector.tensor_mul(sq, z, z)
            else:
                nc.scalar.square(sq, z)
            if s >= NSB - 3:
                pending.append((s, None))
            else:
                m = mpool.tile([P, SB], F16, tag="m", name="m")
                eng = nc.gpsimd if s in POOL_MERGE else nc.vector
                eng.tensor_add(m, sq[:, 0, :], sq[:, 1, :])
                if nsub:
                    # mixed-sign slots live at the front; overwrite them
                    eng.tensor_sub(
                        m[0:nsub, :], sq[0:nsub, 1, :], sq[0:nsub, 0, :]
                    )
                pending.append((s, m))
            if len(pending) > RLAG:
                emit_reduce(*pending.pop(0))
        for item in pending:
            emit_reduce(*item)

    nc.compile()
    return nc


def _get_program():
    nc = _PROGRAMS.get(_BUILD_KEY)
    if nc is None:
        nc = _PROGRAMS[_BUILD_KEY] = _build_program(_BUILD_KEY)
    return nc


def _factor(prec):
    """S = A diag(w) A^T with A[0:128, 128:256] = 0 when well-conditioned
    (schur variant, 3 matmuls), else dense eigh (4). Columns normalized."""
    S = 0.5 * (prec + prec.T)
    S00, S10, S11 = S[:P, :P], S[P:, :P], S[P:, P:]
    l0, Q0 = np.linalg.eigh(S00)
    ok = np.abs(l0).min() > 1e-3
    if ok:
        A10 = S10 @ Q0 @ np.diag(1.0 / l0)
        ok = np.abs(A10).max() < 500.0
    if ok:
        C = S11 - (A10 * l0) @ A10.T
        lc, Qc = np.linalg.eigh(C)
        A = np.zeros((D, D))
        A[:P, :P] = Q0
        A[P:, :P] = A10
        A[P:, P:] = Qc
        w = np.concatenate([l0, lc])
        variant = "schur"
    else:
        w, A = np.linalg.eigh(S)
        variant = "eigh"
    nrm = np.linalg.norm(A, axis=0)
    return variant, A / nrm, -(w * nrm**2)


def _pair(An, wn):
    """Order/rescale columns so chunk0/chunk1 slot k share |weight|; returns
    (A0, A1, wstar, wab, nadd): first nadd slots add, the rest subtract."""
    w0, w1 = wn[:P], wn[P:]
    A0, A1 = An[:, :P], An[:, P:]

    def by_sign(wv, sgn):
        idx = np.where(np.sign(wv) == sgn)[0]
        return idx[np.argsort(-np.abs(wv[idx]))]

    p0, n0 = by_sign(w0, 1), by_sign(w0, -1)
    p1, n1 = by_sign(w1, 1), by_sign(w1, -1)
    kp, kn = min(len(p0), len(p1)), min(len(n0), len(n1))
    # mixed-sign leftovers first (aligned overwrite-sub), then same-sign pairs
    s0 = np.concatenate([p0[kp:], n0[kn:], p0[:kp], n0[:kn]]).astype(int)
    s1 = np.concatenate([n1[kn:], p1[kp:], p1[:kp], n1[:kn]]).astype(int)
    nsub = len(s0) - (kp + kn)
    wa, wb = w0[s0], w1[s1]
    wstar = np.sign(wb) * np.sqrt(np.abs(wa) * np.abs(wb))
    c0 = np.sqrt(np.abs(wa) / np.abs(wstar))
    c1 = np.sqrt(np.abs(wb) / np.abs(wstar))
    # after rescaling, per-chunk weights are +-|wstar| with original signs
    wab = np.stack([np.sign(wa) * np.abs(wstar), np.sign(wb) * np.abs(wstar)], 1)
    return A0[:, s0] * c0[None, :], A1[:, s1] * c1[None, :], wstar, wab, nsub


def _host_inputs(X, mean, prec):
    global _BUILD_KEY
    X = np.ascontiguousarray(np.asarray(X, dtype=np.float32))
    m = np.asarray(mean, dtype=np.float32).reshape(-1)
    Pm = np.asarray(prec, dtype=np.float64)

    variant, An, wn = _factor(Pm)
    A0, A1, wstar, wab, nsub = _pair(An, wn)
    _BUILD_KEY = (variant, int(nsub))

    if variant == "schur":
        chunks = [A0[:P], A0[P:], A1[P:]]
    else:
        chunks = [A0[:P], A0[P:], A1[:P], A1[P:]]
    aw_host = np.concatenate(
        [np.concatenate(chunks, axis=1), wstar[:, None], wab], axis=1
    ).astype(np.float16)
    aw_host = np.ascontiguousarray(aw_host)

    Y = (X - m[None, :]).astype(np.float16)
    Yt = np.ascontiguousarray(Y.T)  # [256, 65536] fp16
    in_maps = [
        {
            "y": np.ascontiguousarray(
                Yt[:, i * NS : (i + 1) * NS].reshape(2, P, NS)
            ),
            "aw": aw_host,
        }
        for i in range(N_CORES)
    ]
    return in_maps


def kernel(X, mean, prec):
    global LAST_EXEC_NS, LAST_RESULTS
    from concourse.bass_utils import run_bass_kernel_spmd

    in_maps = _host_inputs(X, mean, prec)
    nc = _get_program()
    res = run_bass_kernel_spmd(
        nc, in_maps, core_ids=list(range(N_CORES)), trace=TRACE
    )
    LAST_RESULTS = res
    LAST_EXEC_NS = res.exec_time_ns
    out = np.concatenate([res.results[i]["out"] for i in range(N_CORES)])
    return out.astype(np.float32)
